# revision 32
# baseline (speedup 1.0000x reference)
"""Causal self-attention (B=4, T=2048, C=1024, H=16) on 8 TRN2 NeuronCores.

Sharding: tensor-parallel over heads — 2 heads per core. Each core gets the
full x (replicated, bf16 + fp8 copies), its W_attn column slice (q|k|v for
its 2 heads), and its 128-row slice of W_proj; it produces a full-shape
[B*T, C] fp16 partial output which the host sums across cores (b_proj added
on host).

Per-core pipeline (activations feature-on-partition, "transposed"):
  A. x^T loads (host-pretransposed): fp8 for the K/Q projections, bf16
     for V.  Later batches prefetch during the previous batch's compute.
  B. qkv^T[f, t] = W.T @ x^T (+bias); K and Q via fp8 DoubleRow
     (contraction 256 per pass, W upscaled x64 on host so e4m3 stays in
     normal range).  V^T -> vaug = [V_h | 1] slots via one PE transpose
     plus one strided DVE copy per kt chunk.
  C. Per (j, kc), diagonal chunks first (scores before any PV so the PE
     queue never head-of-line blocks on the psy bank handoff): both
     heads' score matmuls pack one [128, 2*512] PSUM tile (row-band
     tile_position); ONE exp ACT over the causally-needed column suffix
     only; causal triangle masked in-place by GpSimd affine_select over
     the first 128 suffix columns (fill=0); per-head [V|1].T @ P^T
     accumulated into one [128, 2, 512] PSUM tile (65 live rows per
     head; row 64 = softmax denominator).
     Normalization is latency-split: an ACT copy (y rows) plus a DVE
     copy (denom row -> partition 0) release the psy banks immediately;
     the reciprocal, GpSimd partition broadcast and the multiplies are
     deferred to the NEXT j boundary so the GpSimd queue never stalls
     ahead of the next j's causal selects.
  D. out = y^T.T @ W_proj per 128-token chunk, drained to fp16 and
     DMA'd out two chunks per descriptor.  Emission is software-
     pipelined: phase D chunks and the NEXT batch's projection groups
     are interleaved at the phase-C j boundaries so the PE stays fed
     while each j's normalize chain completes.

PSUM budget (8 banks): ps_s 2x[128, 2*512] score tiles; ps_yo
1x[128, 2, 512] PV accumulator; ps_io 2x[128, 512] shared by phase-B
projection tiles, V-transposes and phase-D output tiles in strict
emission alternation.
"""

import sys
import numpy as np

if "/opt/trn_rl_repo" not in sys.path:
    sys.path.insert(0, "/opt/trn_rl_repo")

from contextlib import ExitStack

import ml_dtypes
import concourse.bass as bass
import concourse.mybir as mybir
import concourse.tile as tile
from concourse import bacc
from concourse.bass_utils import run_bass_kernel_spmd
from concourse.masks import make_identity

B, T, C, H, D = 4, 2048, 1024, 16, 64
P = 128
NCORES = 8
HPC = H // NCORES          # 2 heads per core
FC = HPC * D               # 128 features per core per q/k/v
NT = B * T                 # 8192 tokens
CC = C // P                # 8 contraction chunks
TJ = 512                   # token tile (free dim) for big matmuls
NQ = T // TJ               # 4 qt chunks per batch
KCH = T // P               # 16 kt chunks per batch
F32 = mybir.dt.float32
F16 = mybir.dt.float16
BF16 = mybir.dt.bfloat16
FP8 = mybir.dt.float8e4
AF = mybir.ActivationFunctionType
ALU = mybir.AluOpType
DR = mybir.MatmulPerfMode.DoubleRow
W_SCALE = 64.0  # host-side upscale of fp8 W slices so e4m3 stays normal-range
Q_FP8 = True   # Q projection via fp8 DoubleRow (cheaper, rel_err ~1.8e-2)

_CACHE = {}


def build_program():
    nc = bacc.Bacc("TRN2", target_bir_lowering=False, debug=False)

    xt_d = nc.dram_tensor("xt", [C, NT], BF16, kind="ExternalInput").ap()
    xt8_d = nc.dram_tensor("xt8", [C, NT], FP8, kind="ExternalInput").ap()
    nqv = FC if Q_FP8 else 2 * FC
    wa_d = nc.dram_tensor("w_qv", [C, nqv], BF16, kind="ExternalInput").ap()
    n8 = 2 * FC if Q_FP8 else FC
    w8_d = nc.dram_tensor("w_k8", [C, n8], FP8, kind="ExternalInput").ap()
    ba_d = nc.dram_tensor("b_attn", [3, FC], F32, kind="ExternalInput").ap()
    wp_d = nc.dram_tensor("w_proj", [FC, C], BF16, kind="ExternalInput").ap()
    out_d = nc.dram_tensor("out", [NT, C], F16, kind="ExternalOutput").ap()

    with tile.TileContext(nc) as tc, ExitStack() as ctx:
        consts = ctx.enter_context(tc.tile_pool(name="consts", bufs=1))
        xt_pool = ctx.enter_context(tc.tile_pool(name="xt", bufs=2))
        qkvt_pool = ctx.enter_context(tc.tile_pool(name="qkvt", bufs=2))
        vaug_pool = ctx.enter_context(tc.tile_pool(name="vaug", bufs=2))
        pt_pool = ctx.enter_context(tc.tile_pool(name="pt", bufs=6))
        sums_pool = ctx.enter_context(tc.tile_pool(name="sums", bufs=2))
        yu_pool = ctx.enter_context(tc.tile_pool(name="yu", bufs=3))
        rbc_pool = ctx.enter_context(tc.tile_pool(name="rbc", bufs=2))
        y_pool = ctx.enter_context(tc.tile_pool(name="y", bufs=8))
        o_pool = ctx.enter_context(tc.tile_pool(name="o", bufs=3))

        ps_s = ctx.enter_context(tc.tile_pool(name="ps_s", bufs=2, space="PSUM"))
        ps_yo = ctx.enter_context(tc.tile_pool(name="ps_yo", bufs=1, space="PSUM"))
        ps_io = ctx.enter_context(tc.tile_pool(name="ps_io", bufs=2, space="PSUM"))

        def io_tile(shape, name):
            # phase-B projection tiles and phase-D output tiles cycle the
            # same two 1-bank ps_io buffers in strict emission alternation.
            return ps_io.tile(shape, F32, name=name, tag="ps_io")

        # --- constants needed by the first projection groups ---
        wa_r = wa_d.rearrange("(cc p) f -> p cc f", p=P)
        w8_r = w8_d.rearrange("(cc p) f -> p cc f", p=P)
        w8_sb = consts.tile([P, CC, n8], FP8)
        nc.sync.dma_start(w8_sb[:], w8_r)
        w_qv_sb = consts.tile([P, CC, nqv], BF16)
        nc.sync.dma_start(w_qv_sb[:], wa_r)
        bias_sb = consts.tile([P, 3], F32)
        nc.scalar.dma_start(bias_sb[:], ba_d.rearrange("f p -> p f"))
        # --- later-needed constants, emitted after batch 0's x loads ---
        wp_sb = consts.tile([P, C], BF16)
        ident = consts.tile([P, P], F32)
        identb = consts.tile([P, P], BF16)
        ones_st = consts.tile([P, 1], F32)
        ones_b = consts.tile([P, 1], BF16)

        def emit_late_consts():
            nc.sync.dma_start(wp_sb[:], wp_d)
            make_identity(nc, ident[:])
            nc.vector.tensor_copy(out=identb[:], in_=ident[:])
            nc.vector.memset(ones_st[:], 1.0)
            nc.vector.tensor_copy(out=ones_b[:], in_=ones_st[:])

        # Phase D of batch b is deferred and emitted interleaved with phase
        # B of batch b+1.  Each pending entry is one 128-token chunk; chunk
        # pairs share an ost tile so the output DMA moves 256 rows at once.
        pending_d = []

        def emit_d_chunk(j0row, ysb, tb, ost, split_drain=False):
            par = tb % 2
            ts = slice(tb * P, (tb + 1) * P)
            for cn in range(C // TJ):
                pso = io_tile([P, TJ], "pso")
                nc.tensor.matmul(
                    pso[:],
                    ysb[:, ts],
                    wp_sb[:, cn * TJ:(cn + 1) * TJ],
                    start=True,
                    stop=True,
                )
                osl = ost[:, par, cn * TJ:(cn + 1) * TJ]
                if split_drain and cn == 0:
                    nc.scalar.copy(osl, pso[:])
                else:
                    nc.vector.tensor_copy(out=osl, in_=pso[:])
            if par == 1:
                r0 = j0row + (tb - 1) * P
                nc.sync.dma_start(
                    out_d[r0:r0 + 2 * P, :].rearrange("(two p) c -> p two c", p=P),
                    ost[:],
                )

        def emit_d_pair(j0row, ysb, pair, split_drain=False):
            ost = o_pool.tile([P, 2, C], F16, name="ost", tag="ost")
            emit_d_chunk(j0row, ysb, 2 * pair, ost, split_drain)
            emit_d_chunk(j0row, ysb, 2 * pair + 1, ost, split_drain)

        xt8_r = xt8_d.rearrange("(cc p) t -> p cc t", p=P)

        def alloc_x_tiles():
            x8 = [xt_pool.tile([P, 2, T], FP8, name=f"xt8_{c2}",
                               tag=f"xt8_{c2}") for c2 in range(CC // 2)]
            xb = [xt_pool.tile([P, T], BF16, name=f"xt{cc}",
                               tag=f"xt{cc}") for cc in range(CC)]
            return xb, x8

        def emit_prefetch(bn, tiles):
            xb, x8 = tiles
            nt0 = bn * T
            for c2 in range(CC // 2):
                nc.sync.dma_start(x8[c2][:],
                                  xt8_r[:, 2 * c2:2 * c2 + 2, nt0:nt0 + T])
            for cc in range(CC):
                nc.sync.dma_start(xb[cc][:],
                                  xt_d[cc * P:(cc + 1) * P, nt0:nt0 + T])

        def emit_proj_group(tj, f, xtiles, qkvt, vaug):
            # one (tj, f) projection group; after V (f==2), transpose the
            # tj's four V chunks into vaug
            xtb, xt8b = xtiles
            tjs = slice(tj * TJ, (tj + 1) * TJ)
            psf = io_tile([P, TJ], "psf")
            if f in ((1, 0) if Q_FP8 else (1,)):
                w8o = 0 if f == 1 else FC
                for c2 in range(CC // 2):
                    nc.tensor.matmul(
                        psf[:],
                        w8_sb[:, 2 * c2:2 * c2 + 2, w8o:w8o + FC],
                        xt8b[c2][:, :, tjs],
                        start=(c2 == 0),
                        stop=(c2 == CC // 2 - 1),
                        perf_mode=DR,
                    )
                nc.vector.tensor_scalar(
                    out=qkvt[:, f, tjs], in0=psf[:],
                    scalar1=float(1.0 / W_SCALE),
                    scalar2=bias_sb[:, f:f + 1],
                    op0=ALU.mult, op1=ALU.add,
                )
            else:
                wo = 0 if (f == 0 and not Q_FP8) else nqv - FC
                for cc in range(CC):
                    nc.tensor.matmul(
                        psf[:],
                        w_qv_sb[:, cc, wo:wo + FC],
                        xtb[cc][:, tjs],
                        start=(cc == 0),
                        stop=(cc == CC - 1),
                    )
                nc.vector.tensor_scalar_add(
                    qkvt[:, f, tjs], psf[:], bias_sb[:, f:f + 1]
                )
            if f == 2:
                for kc in range(4 * tj, 4 * tj + 4):
                    pst = ps_io.tile([P, P], BF16, name="pst", tag="ps_io")
                    nc.tensor.transpose(
                        pst[:], qkvt[:, 2, kc * P:(kc + 1) * P], identb[:])
                    nc.vector.tensor_copy(
                        out=vaug[:, kc, :, 0:D], in_=pst[:])

        # ---- batch 0: phase A + full phase B inline ----
        x_tiles = [None] * (B + 1)
        qkvts = [None] * B
        vaugs = [None] * B
        x_tiles[0] = alloc_x_tiles()
        xtb0, xt8b0 = x_tiles[0]
        for half in range(2):
            hs = slice(half * (T // 2), (half + 1) * (T // 2))
            for c2 in range(CC // 2):
                q = nc.scalar if c2 % 2 else nc.sync
                q.dma_start(xt8b0[c2][:, :, hs], xt8_r[:, 2 * c2:2 * c2 + 2, hs])
            for cc in range(CC):
                q = nc.scalar if cc % 2 else nc.sync
                q.dma_start(xtb0[cc][:, hs], xt_d[cc * P:(cc + 1) * P, hs])
            if half == 0:
                emit_late_consts()
        del xtb0, xt8b0
        qkvts[0] = qkvt_pool.tile([P, 3, T], BF16, name="qkvt")
        vaugs[0] = vaug_pool.tile([P, KCH, HPC, D + 1], BF16, name="vaug")
        nc.vector.tensor_copy(
            out=vaugs[0][:, :, :, D:D + 1],
            in_=ones_b[:, None, None, :].to_broadcast((P, KCH, HPC, 1)),
        )
        for tj in range(NQ):
            for f in (1, 0, 2):
                emit_proj_group(tj, f, x_tiles[0], qkvts[0], vaugs[0])
        x_tiles[1] = alloc_x_tiles()
        emit_prefetch(1, x_tiles[1])

        # pending output chunks: (row0, ysb, pair) emitted at later j
        # boundaries so their matmuls never head-of-line block the PE
        pending_d = []
        # deferred second half of the softmax normalization
        pending_norm = []

        def emit_norm_finish(yu, recip, row0):
            rbc = rbc_pool.tile([D, HPC, TJ], F32, name="rbc", tag="rbc")
            nc.gpsimd.partition_broadcast(rbc[:], recip[:])
            ysb = y_pool.tile([P, TJ], BF16, name="ysb")
            for h in range(HPC):
                nc.vector.tensor_mul(
                    out=ysb[h * D:(h + 1) * D, :],
                    in0=yu[:D, h, :],
                    in1=rbc[:, h, :],
                )
            for pair in range(TJ // P // 2):
                pending_d.append((row0, ysb, pair))
        PACE = (2, 3, 3, 4)   # B(b+1) proj groups per j boundary
        PACE0 = (0, 4, 4, 4)  # batch 0: defer past the tight first boundary

        for b in range(B):
            t0 = b * T
            qkvt = qkvts[b]
            vaug = vaugs[b]
            b_work = []
            if b + 1 < B:
                qkvts[b + 1] = qkvt_pool.tile([P, 3, T], BF16, name="qkvt")
                vaugs[b + 1] = vaug_pool.tile([P, KCH, HPC, D + 1], BF16, name="vaug")
                nc.vector.tensor_copy(
                    out=vaugs[b + 1][:, :, :, D:D + 1],
                    in_=ones_b[:, None, None, :].to_broadcast((P, KCH, HPC, 1)),
                )
                b_work = [(tj, f) for tj in range(NQ) for f in (1, 0, 2)]

            # ---- phase C ----
            for j in range(NQ):
                nkc = 4 * j + 4
                psy = ps_yo.tile([P, HPC, TJ], F32, name="psy", tag="psy")
                kc_order = [4 * j + 3, 4 * j + 2, 4 * j + 1, 4 * j] + \
                    list(range(0, 4 * j))

                def emit_scores(kc):
                    # scores + exp (+ causal select) over the causally
                    # needed column suffix; returns (pt tile, suffix)
                    r = kc - 4 * j
                    cs = slice(r * P, TJ) if r > 0 else slice(0, TJ)
                    pss = ps_s.tile([P, HPC, TJ], F32, name="pss", tag="pss")
                    for h in range(HPC):
                        hd = slice(h * D, (h + 1) * D)
                        nc.tensor.matmul(
                            pss[:, h, cs],
                            qkvt[hd, 1, kc * P:(kc + 1) * P],
                            qkvt[hd, 0, j * TJ + cs.start:(j + 1) * TJ],
                            start=True,
                            stop=True,
                            tile_position=(h * D, 0),
                        )
                    pt = pt_pool.tile([P, HPC, TJ], BF16, name="pt", tag="pt")
                    nc.scalar.activation(
                        pt[:, :, cs], pss[:, :, cs], AF.Exp,
                        bias=0.0, scale=float(1.0 / np.sqrt(D)),
                    )
                    if r >= 0:
                        # keep pt[kt, h, q] only where q >= kt (both heads);
                        # only the first 128 suffix columns can be masked
                        # (kt < 128), so the select stops there
                        ms = slice(cs.start, cs.start + P)
                        nc.gpsimd.affine_select(
                            out=pt[:, :, ms],
                            in_=pt[:, :, ms],
                            compare_op=ALU.is_ge,
                            fill=0.0,
                            base=0,
                            pattern=[[0, HPC], [1, P]],
                            channel_multiplier=-1,
                        )
                    return pt, cs

                def emit_pv(kc, pt, cs, first, last):
                    for h in range(HPC):
                        nc.tensor.matmul(
                            psy[:D + 1, h, cs],
                            vaug[:, kc, h, :],
                            pt[:, h, cs],
                            start=first,
                            stop=last,
                        )

                # diagonal chunks: all scores first (their PVs are the
                # first psy writers and may briefly wait on the previous
                # j's psy release — keep the PE queue fed with scores)
                diag = kc_order[:4]
                diag_pt = [emit_scores(kc) for kc in diag]
                for i, kc in enumerate(diag):
                    pt, cs = diag_pt[i]
                    emit_pv(kc, pt, cs, i == 0, i == nkc - 1)
                if b == B - 1:
                    # last batch has no next-batch projection work: fill the
                    # PE mid-loop instead of at the starved boundary (the
                    # norm-finish broadcast queues AFTER this j's selects,
                    # so it cannot block them)
                    if pending_norm:
                        emit_norm_finish(*pending_norm.pop(0))
                    for _ in range(2):
                        if pending_d:
                            emit_d_pair(*pending_d.pop(0), split_drain=True)
                for i, kc in enumerate(kc_order[4:]):
                    pt, cs = emit_scores(kc)
                    emit_pv(kc, pt, cs, False, i == nkc - 5)

                # normalization, first half: drain psy fast on two
                # parallel engines (body rows -> yu on ACT, denom row 64 ->
                # sums on DVE) so the next j's PV can take the banks; the
                # partition broadcast and the multiplies are deferred one
                # boundary so the GpSimd queue never waits ahead of the
                # next j's causal selects.  sums lands on partition 0 — the
                # custom-DVE reciprocal mishandles offset partitions.
                yu = yu_pool.tile([D, HPC, TJ], F32, name="yu", tag="yu")
                nc.scalar.copy(yu[:], psy[:D, :, :])
                sums = sums_pool.tile([1, HPC, TJ], F32, name="sums",
                                      tag="sums")
                nc.vector.tensor_copy(out=sums[:], in_=psy[D:D + 1, :, :])
                recip = sums_pool.tile([1, HPC, TJ], F32, name="recip",
                                       tag="recip")
                nc.vector.reciprocal_approx_fast(out=recip[:], in_=sums[:])

                # ---- j-boundary fill work (keeps PE fed while the
                #      normalize chain for this j completes) ----
                npop = 2 if b == B - 1 else (1 if b == B - 2 else 2)
                for _ in range(npop):
                    if pending_d:
                        emit_d_pair(*pending_d.pop(0),
                                    split_drain=(b == B - 1))
                if b != B - 1 and pending_norm:
                    emit_norm_finish(*pending_norm.pop(0))
                pending_norm.append((yu, recip, t0 + j * TJ))
                for _ in range((PACE0 if b == 0 else PACE)[j]):
                    if b_work:
                        tj, f = b_work.pop(0)
                        emit_proj_group(tj, f, x_tiles[b + 1],
                                        qkvts[b + 1], vaugs[b + 1])

            while b_work:
                tj, f = b_work.pop(0)
                emit_proj_group(tj, f, x_tiles[b + 1], qkvts[b + 1],
                                vaugs[b + 1])
            if b + 2 < B:
                x_tiles[b + 2] = alloc_x_tiles()
                emit_prefetch(b + 2, x_tiles[b + 2])

        # tail: remaining normalize + output chunks of the last batch
        while pending_norm:
            emit_norm_finish(*pending_norm.pop(0))
        while pending_d:
            emit_d_pair(*pending_d.pop(0), split_drain=True)

    nc.compile()
    return nc


def make_in_maps(x, W_attn, b_attn, W_proj):
    x_flat = np.asarray(x, dtype=np.float32).reshape(NT, C)
    xt = np.ascontiguousarray(x_flat.T)
    xt_bf = xt.astype(ml_dtypes.bfloat16)
    xt_f8 = xt.astype(ml_dtypes.float8_e4m3)
    W_attn = np.asarray(W_attn, dtype=np.float32)
    b_attn = np.asarray(b_attn, dtype=np.float32)
    W_proj = np.asarray(W_proj, dtype=np.float32)
    in_maps = []
    for core in range(NCORES):
        lo = core * FC
        cols = np.concatenate(
            [np.arange(lo, lo + FC) + k * C for k in range(3)]
        )
        w_slice = W_attn[:, cols]
        wq, wk, wv = w_slice[:, :FC], w_slice[:, FC:2 * FC], w_slice[:, 2 * FC:]
        if Q_FP8:
            w_qv = wv
            w_k8 = np.concatenate([wk, wq], axis=1)
        else:
            w_qv = np.concatenate([wq, wv], axis=1)
            w_k8 = wk
        in_maps.append({
            "xt": xt_bf,
            "xt8": xt_f8,
            "w_qv": np.ascontiguousarray(w_qv.astype(ml_dtypes.bfloat16)),
            "w_k8": np.ascontiguousarray(
                (w_k8 * W_SCALE).astype(ml_dtypes.float8_e4m3)),
            "b_attn": np.ascontiguousarray(b_attn[cols].reshape(3, FC)),
            "w_proj": np.ascontiguousarray(
                W_proj[lo:lo + FC, :].astype(ml_dtypes.bfloat16)),
        })
    return in_maps


def kernel(x, W_attn, b_attn, W_proj, b_proj, **run_kwargs):
    if "nc" not in _CACHE:
        _CACHE["nc"] = build_program()
    nc = _CACHE["nc"]
    in_maps = make_in_maps(x, W_attn, b_attn, W_proj)
    res = run_bass_kernel_spmd(nc, in_maps, core_ids=list(range(NCORES)), **run_kwargs)
    _CACHE["last_results"] = res
    total = np.zeros((NT, C), dtype=np.float32)
    for r in res.results:
        total += np.asarray(r["out"], dtype=np.float32)
    total += np.asarray(b_proj, dtype=np.float32)[None, :]
    return total.reshape(B, T, C)


# revision 33
# speedup vs baseline: 1.0311x; 1.0311x over previous
"""Causal self-attention (B=4, T=2048, C=1024, H=16) on 8 TRN2 NeuronCores.

Sharding: tensor-parallel over heads — 2 heads per core. Each core gets the
full x (replicated, bf16 + fp8 copies), its W_attn column slice (q|k|v for
its 2 heads), and its 128-row slice of W_proj; it produces a full-shape
[B*T, C] fp16 partial output which the host sums across cores (b_proj added
on host).

Per-core pipeline (activations feature-on-partition, "transposed"):
  A. x^T loads (host-pretransposed): fp8 for the K/Q projections, bf16
     for V.  Later batches prefetch during the previous batch's compute.
  B. qkv^T[f, t] = W.T @ x^T (+bias); K and Q via fp8 DoubleRow
     (contraction 256 per pass, W upscaled x64 on host so e4m3 stays in
     normal range).  V^T -> vaug = [V_h | 1] slots via one PE transpose
     plus one strided DVE copy per kt chunk.
  C. Per (j, kc), diagonal chunks first (scores before any PV so the PE
     queue never head-of-line blocks on the psy bank handoff): both
     heads' score matmuls pack one [128, 2*512] PSUM tile (row-band
     tile_position); ONE exp ACT over the causally-needed column suffix
     only; causal triangle masked in-place by GpSimd affine_select over
     the first 128 suffix columns (fill=0); per-head [V|1].T @ P^T
     accumulated into one [128, 2, 512] PSUM tile (65 live rows per
     head; row 64 = softmax denominator).
     Normalization is latency-split: an ACT copy (y rows) plus a DVE
     copy (denom row -> partition 0) release the psy banks immediately;
     the reciprocal, GpSimd partition broadcast and the multiplies are
     deferred to the NEXT j boundary so the GpSimd queue never stalls
     ahead of the next j's causal selects.
  D. out = y^T.T @ W_proj per 128-token chunk, drained to fp16 and
     DMA'd out two chunks per descriptor.  Emission is software-
     pipelined: phase D chunks and the NEXT batch's projection groups
     are interleaved at the phase-C j boundaries so the PE stays fed
     while each j's normalize chain completes.

PSUM budget (8 banks): ps_s 2x[128, 2*512] score tiles; ps_yo
1x[128, 2, 512] PV accumulator; ps_io 2x[128, 512] shared by phase-B
projection tiles, V-transposes and phase-D output tiles in strict
emission alternation.
"""

import sys
import numpy as np

if "/opt/trn_rl_repo" not in sys.path:
    sys.path.insert(0, "/opt/trn_rl_repo")

from contextlib import ExitStack

import ml_dtypes
import concourse.bass as bass
import concourse.mybir as mybir
import concourse.tile as tile
from concourse import bacc
from concourse.bass_utils import run_bass_kernel_spmd
from concourse.masks import make_identity

B, T, C, H, D = 4, 2048, 1024, 16, 64
P = 128
NCORES = 8
HPC = H // NCORES          # 2 heads per core
FC = HPC * D               # 128 features per core per q/k/v
NT = B * T                 # 8192 tokens
CC = C // P                # 8 contraction chunks
TJ = 512                   # token tile (free dim) for big matmuls
NQ = T // TJ               # 4 qt chunks per batch
KCH = T // P               # 16 kt chunks per batch
F32 = mybir.dt.float32
F16 = mybir.dt.float16
BF16 = mybir.dt.bfloat16
FP8 = mybir.dt.float8e4
AF = mybir.ActivationFunctionType
ALU = mybir.AluOpType
DR = mybir.MatmulPerfMode.DoubleRow
W_SCALE = 64.0  # host-side upscale of fp8 W slices so e4m3 stays normal-range
Q_FP8 = True   # Q projection via fp8 DoubleRow (cheaper, rel_err ~1.8e-2)

_CACHE = {}


def build_program():
    nc = bacc.Bacc("TRN2", target_bir_lowering=False, debug=False)

    xt_d = nc.dram_tensor("xt", [C, NT], BF16, kind="ExternalInput").ap()
    xt8_d = nc.dram_tensor("xt8", [C, NT], FP8, kind="ExternalInput").ap()
    nqv = FC if Q_FP8 else 2 * FC
    wa_d = nc.dram_tensor("w_qv", [C, nqv], BF16, kind="ExternalInput").ap()
    n8 = 2 * FC if Q_FP8 else FC
    w8_d = nc.dram_tensor("w_k8", [C, n8], FP8, kind="ExternalInput").ap()
    ba_d = nc.dram_tensor("b_attn", [3, FC], F32, kind="ExternalInput").ap()
    wp_d = nc.dram_tensor("w_proj", [FC, C], BF16, kind="ExternalInput").ap()
    out_d = nc.dram_tensor("out", [NT, C], F16, kind="ExternalOutput").ap()

    with tile.TileContext(nc) as tc, ExitStack() as ctx:
        consts = ctx.enter_context(tc.tile_pool(name="consts", bufs=1))
        xt_pool = ctx.enter_context(tc.tile_pool(name="xt", bufs=2))
        qkvt_pool = ctx.enter_context(tc.tile_pool(name="qkvt", bufs=2))
        vaug_pool = ctx.enter_context(tc.tile_pool(name="vaug", bufs=2))
        pt_pool = ctx.enter_context(tc.tile_pool(name="pt", bufs=6))
        sums_pool = ctx.enter_context(tc.tile_pool(name="sums", bufs=2))
        yu_pool = ctx.enter_context(tc.tile_pool(name="yu", bufs=3))
        rbc_pool = ctx.enter_context(tc.tile_pool(name="rbc", bufs=2))
        y_pool = ctx.enter_context(tc.tile_pool(name="y", bufs=8))
        o_pool = ctx.enter_context(tc.tile_pool(name="o", bufs=3))

        ps_s = ctx.enter_context(tc.tile_pool(name="ps_s", bufs=2, space="PSUM"))
        ps_yo = ctx.enter_context(tc.tile_pool(name="ps_yo", bufs=1, space="PSUM"))
        ps_io = ctx.enter_context(tc.tile_pool(name="ps_io", bufs=2, space="PSUM"))

        def io_tile(shape, name):
            # phase-B projection tiles and phase-D output tiles cycle the
            # same two 1-bank ps_io buffers in strict emission alternation.
            return ps_io.tile(shape, F32, name=name, tag="ps_io")

        # --- constants needed by the first projection groups ---
        wa_r = wa_d.rearrange("(cc p) f -> p cc f", p=P)
        w8_r = w8_d.rearrange("(cc p) f -> p cc f", p=P)
        w8_sb = consts.tile([P, CC, n8], FP8)
        nc.sync.dma_start(w8_sb[:], w8_r)
        w_qv_sb = consts.tile([P, CC, nqv], BF16)
        nc.sync.dma_start(w_qv_sb[:], wa_r)
        bias_sb = consts.tile([P, 3], F32)
        nc.scalar.dma_start(bias_sb[:], ba_d.rearrange("f p -> p f"))
        # --- later-needed constants, emitted after batch 0's x loads ---
        wp_sb = consts.tile([P, C], BF16)
        ident = consts.tile([P, P], F32)
        identb = consts.tile([P, P], BF16)
        ones_st = consts.tile([P, 1], F32)
        ones_b = consts.tile([P, 1], BF16)

        def emit_late_consts():
            nc.sync.dma_start(wp_sb[:], wp_d)
            make_identity(nc, ident[:])
            nc.vector.tensor_copy(out=identb[:], in_=ident[:])
            nc.vector.memset(ones_st[:], 1.0)
            nc.vector.tensor_copy(out=ones_b[:], in_=ones_st[:])

        # Phase D of batch b is deferred and emitted interleaved with phase
        # B of batch b+1.  Each pending entry is one 128-token chunk; chunk
        # pairs share an ost tile so the output DMA moves 256 rows at once.
        pending_d = []

        def emit_d_chunk(j0row, ysb, tb, ost, split_drain=False):
            par = tb % 2
            ts = slice(tb * P, (tb + 1) * P)
            for cn in range(C // TJ):
                pso = io_tile([P, TJ], "pso")
                nc.tensor.matmul(
                    pso[:],
                    ysb[:, ts],
                    wp_sb[:, cn * TJ:(cn + 1) * TJ],
                    start=True,
                    stop=True,
                )
                osl = ost[:, par, cn * TJ:(cn + 1) * TJ]
                if split_drain and cn == 0:
                    nc.scalar.copy(osl, pso[:])
                else:
                    nc.vector.tensor_copy(out=osl, in_=pso[:])
            if par == 1:
                r0 = j0row + (tb - 1) * P
                nc.sync.dma_start(
                    out_d[r0:r0 + 2 * P, :].rearrange("(two p) c -> p two c", p=P),
                    ost[:],
                )

        def emit_d_pair(j0row, ysb, pair, split_drain=False):
            ost = o_pool.tile([P, 2, C], F16, name="ost", tag="ost")
            emit_d_chunk(j0row, ysb, 2 * pair, ost, split_drain)
            emit_d_chunk(j0row, ysb, 2 * pair + 1, ost, split_drain)

        xt8_r = xt8_d.rearrange("(cc p) t -> p cc t", p=P)

        def alloc_x_tiles():
            x8 = [xt_pool.tile([P, 2, T], FP8, name=f"xt8_{c2}",
                               tag=f"xt8_{c2}") for c2 in range(CC // 2)]
            xb = [xt_pool.tile([P, T], BF16, name=f"xt{cc}",
                               tag=f"xt{cc}") for cc in range(CC)]
            return xb, x8

        def emit_prefetch(bn, tiles):
            xb, x8 = tiles
            nt0 = bn * T
            for c2 in range(CC // 2):
                nc.sync.dma_start(x8[c2][:],
                                  xt8_r[:, 2 * c2:2 * c2 + 2, nt0:nt0 + T])
            for cc in range(CC):
                nc.sync.dma_start(xb[cc][:],
                                  xt_d[cc * P:(cc + 1) * P, nt0:nt0 + T])

        def emit_proj_group(tj, f, xtiles, qkvt, vaug):
            # one (tj, f) projection group; after V (f==2), transpose the
            # tj's four V chunks into vaug
            xtb, xt8b = xtiles
            tjs = slice(tj * TJ, (tj + 1) * TJ)
            psf = io_tile([P, TJ], "psf")
            if f in ((1, 0) if Q_FP8 else (1,)):
                w8o = 0 if f == 1 else FC
                for c2 in range(CC // 2):
                    nc.tensor.matmul(
                        psf[:],
                        w8_sb[:, 2 * c2:2 * c2 + 2, w8o:w8o + FC],
                        xt8b[c2][:, :, tjs],
                        start=(c2 == 0),
                        stop=(c2 == CC // 2 - 1),
                        perf_mode=DR,
                    )
                nc.vector.tensor_scalar(
                    out=qkvt[:, f, tjs], in0=psf[:],
                    scalar1=float(1.0 / W_SCALE),
                    scalar2=bias_sb[:, f:f + 1],
                    op0=ALU.mult, op1=ALU.add,
                )
            else:
                wo = 0 if (f == 0 and not Q_FP8) else nqv - FC
                for cc in range(CC):
                    nc.tensor.matmul(
                        psf[:],
                        w_qv_sb[:, cc, wo:wo + FC],
                        xtb[cc][:, tjs],
                        start=(cc == 0),
                        stop=(cc == CC - 1),
                    )
                nc.vector.tensor_scalar_add(
                    qkvt[:, f, tjs], psf[:], bias_sb[:, f:f + 1]
                )
            if f == 2:
                for kc in range(4 * tj, 4 * tj + 4):
                    pst = ps_io.tile([P, P], BF16, name="pst", tag="ps_io")
                    nc.tensor.transpose(
                        pst[:], qkvt[:, 2, kc * P:(kc + 1) * P], identb[:])
                    nc.vector.tensor_copy(
                        out=vaug[:, kc, :, 0:D], in_=pst[:])

        # ---- batch 0: phase A + full phase B inline ----
        x_tiles = [None] * (B + 1)
        qkvts = [None] * B
        vaugs = [None] * B
        x_tiles[0] = alloc_x_tiles()
        xtb0, xt8b0 = x_tiles[0]
        for half in range(2):
            hs = slice(half * (T // 2), (half + 1) * (T // 2))
            for c2 in range(CC // 2):
                q = nc.scalar if c2 % 2 else nc.sync
                q.dma_start(xt8b0[c2][:, :, hs], xt8_r[:, 2 * c2:2 * c2 + 2, hs])
            for cc in range(CC):
                q = nc.scalar if cc % 2 else nc.sync
                q.dma_start(xtb0[cc][:, hs], xt_d[cc * P:(cc + 1) * P, hs])
            if half == 0:
                emit_late_consts()
        del xtb0, xt8b0
        qkvts[0] = qkvt_pool.tile([P, 3, T], BF16, name="qkvt")
        vaugs[0] = vaug_pool.tile([P, KCH, HPC, D + 1], BF16, name="vaug")
        nc.vector.tensor_copy(
            out=vaugs[0][:, :, :, D:D + 1],
            in_=ones_b[:, None, None, :].to_broadcast((P, KCH, HPC, 1)),
        )
        for tj in range(NQ):
            for f in (1, 0, 2):
                emit_proj_group(tj, f, x_tiles[0], qkvts[0], vaugs[0])
        x_tiles[1] = alloc_x_tiles()
        emit_prefetch(1, x_tiles[1])

        # pending output chunks: (row0, ysb, pair) emitted at later j
        # boundaries so their matmuls never head-of-line block the PE
        pending_d = []
        # deferred second half of the softmax normalization
        pending_norm = []

        def emit_norm_finish(yu, recip, row0):
            rbc = rbc_pool.tile([D, HPC, TJ], F32, name="rbc", tag="rbc")
            nc.gpsimd.partition_broadcast(rbc[:], recip[:])
            ysb = y_pool.tile([P, TJ], BF16, name="ysb")
            for h in range(HPC):
                nc.vector.tensor_mul(
                    out=ysb[h * D:(h + 1) * D, :],
                    in0=yu[:D, h, :],
                    in1=rbc[:, h, :],
                )
            for pair in range(TJ // P // 2):
                pending_d.append((row0, ysb, pair))
        PACE = (2, 3, 3, 4)   # B(b+1) proj groups per j boundary
        PACE0 = (0, 4, 4, 4)  # batch 0: defer past the tight first boundary

        for b in range(B):
            t0 = b * T
            qkvt = qkvts[b]
            vaug = vaugs[b]
            b_work = []
            if b + 1 < B:
                qkvts[b + 1] = qkvt_pool.tile([P, 3, T], BF16, name="qkvt")
                vaugs[b + 1] = vaug_pool.tile([P, KCH, HPC, D + 1], BF16, name="vaug")
                nc.vector.tensor_copy(
                    out=vaugs[b + 1][:, :, :, D:D + 1],
                    in_=ones_b[:, None, None, :].to_broadcast((P, KCH, HPC, 1)),
                )
                b_work = [(tj, f) for tj in range(NQ) for f in (1, 0, 2)]

            # ---- phase C ----
            for j in range(NQ):
                nkc = 4 * j + 4
                psy = ps_yo.tile([P, HPC, TJ], F32, name="psy", tag="psy")
                kc_order = [4 * j + 3, 4 * j + 2, 4 * j + 1, 4 * j] + \
                    list(range(0, 4 * j))

                def emit_scores(kc):
                    # scores + exp (+ causal select) over the causally
                    # needed column suffix; returns (pt tile, suffix)
                    r = kc - 4 * j
                    cs = slice(r * P, TJ) if r > 0 else slice(0, TJ)
                    pss = ps_s.tile([P, HPC, TJ], F32, name="pss", tag="pss")
                    for h in range(HPC):
                        hd = slice(h * D, (h + 1) * D)
                        nc.tensor.matmul(
                            pss[:, h, cs],
                            qkvt[hd, 1, kc * P:(kc + 1) * P],
                            qkvt[hd, 0, j * TJ + cs.start:(j + 1) * TJ],
                            start=True,
                            stop=True,
                            tile_position=(h * D, 0),
                        )
                    pt = pt_pool.tile([P, HPC, TJ], BF16, name="pt", tag="pt")
                    nc.scalar.activation(
                        pt[:, :, cs], pss[:, :, cs], AF.Exp,
                        bias=0.0, scale=float(1.0 / np.sqrt(D)),
                    )
                    if r >= 0:
                        # keep pt[kt, h, q] only where q >= kt (both heads);
                        # only the first 128 suffix columns can be masked
                        # (kt < 128), so the select stops there
                        ms = slice(cs.start, cs.start + P)
                        nc.gpsimd.affine_select(
                            out=pt[:, :, ms],
                            in_=pt[:, :, ms],
                            compare_op=ALU.is_ge,
                            fill=0.0,
                            base=0,
                            pattern=[[0, HPC], [1, P]],
                            channel_multiplier=-1,
                        )
                    return pt, cs

                def emit_pv(kc, pt, cs, first, last):
                    for h in range(HPC):
                        nc.tensor.matmul(
                            psy[:D + 1, h, cs],
                            vaug[:, kc, h, :],
                            pt[:, h, cs],
                            start=first,
                            stop=last,
                        )

                # diagonal chunks: all scores first (their PVs are the
                # first psy writers and may briefly wait on the previous
                # j's psy release — keep the PE queue fed with scores)
                diag = kc_order[:4]
                diag_pt = [emit_scores(kc) for kc in diag]
                for i, kc in enumerate(diag):
                    pt, cs = diag_pt[i]
                    emit_pv(kc, pt, cs, i == 0, i == nkc - 1)
                for i, kc in enumerate(kc_order[4:]):
                    pt, cs = emit_scores(kc)
                    emit_pv(kc, pt, cs, False, i == nkc - 5)

                # normalization, first half: drain psy fast on two
                # parallel engines (body rows -> yu on ACT, denom row 64 ->
                # sums on DVE) so the next j's PV can take the banks; the
                # partition broadcast and the multiplies are deferred one
                # boundary so the GpSimd queue never waits ahead of the
                # next j's causal selects.  sums lands on partition 0 — the
                # custom-DVE reciprocal mishandles offset partitions.
                yu = yu_pool.tile([D, HPC, TJ], F32, name="yu", tag="yu")
                nc.scalar.copy(yu[:], psy[:D, :, :])
                sums = sums_pool.tile([1, HPC, TJ], F32, name="sums",
                                      tag="sums")
                nc.vector.tensor_copy(out=sums[:], in_=psy[D:D + 1, :, :])
                recip = sums_pool.tile([1, HPC, TJ], F32, name="recip",
                                       tag="recip")
                nc.vector.reciprocal_approx_fast(out=recip[:], in_=sums[:])

                # ---- j-boundary fill work (keeps PE fed while the
                #      normalize chain for this j completes) ----
                for _ in range(4 if b == B - 1 else 2):
                    if pending_d:
                        emit_d_pair(*pending_d.pop(0),
                                    split_drain=(b == B - 1))
                if pending_norm:
                    emit_norm_finish(*pending_norm.pop(0))
                pending_norm.append((yu, recip, t0 + j * TJ))
                for _ in range((PACE0 if b == 0 else PACE)[j]):
                    if b_work:
                        tj, f = b_work.pop(0)
                        emit_proj_group(tj, f, x_tiles[b + 1],
                                        qkvts[b + 1], vaugs[b + 1])

            while b_work:
                tj, f = b_work.pop(0)
                emit_proj_group(tj, f, x_tiles[b + 1], qkvts[b + 1],
                                vaugs[b + 1])
            if b + 2 < B:
                x_tiles[b + 2] = alloc_x_tiles()
                emit_prefetch(b + 2, x_tiles[b + 2])

        # tail: remaining normalize + output chunks of the last batch
        while pending_norm:
            emit_norm_finish(*pending_norm.pop(0))
        while pending_d:
            emit_d_pair(*pending_d.pop(0), split_drain=True)

    nc.compile()
    return nc


def make_in_maps(x, W_attn, b_attn, W_proj):
    x_flat = np.asarray(x, dtype=np.float32).reshape(NT, C)
    xt = np.ascontiguousarray(x_flat.T)
    xt_bf = xt.astype(ml_dtypes.bfloat16)
    xt_f8 = xt.astype(ml_dtypes.float8_e4m3)
    W_attn = np.asarray(W_attn, dtype=np.float32)
    b_attn = np.asarray(b_attn, dtype=np.float32)
    W_proj = np.asarray(W_proj, dtype=np.float32)
    in_maps = []
    for core in range(NCORES):
        lo = core * FC
        cols = np.concatenate(
            [np.arange(lo, lo + FC) + k * C for k in range(3)]
        )
        w_slice = W_attn[:, cols]
        wq, wk, wv = w_slice[:, :FC], w_slice[:, FC:2 * FC], w_slice[:, 2 * FC:]
        if Q_FP8:
            w_qv = wv
            w_k8 = np.concatenate([wk, wq], axis=1)
        else:
            w_qv = np.concatenate([wq, wv], axis=1)
            w_k8 = wk
        in_maps.append({
            "xt": xt_bf,
            "xt8": xt_f8,
            "w_qv": np.ascontiguousarray(w_qv.astype(ml_dtypes.bfloat16)),
            "w_k8": np.ascontiguousarray(
                (w_k8 * W_SCALE).astype(ml_dtypes.float8_e4m3)),
            "b_attn": np.ascontiguousarray(b_attn[cols].reshape(3, FC)),
            "w_proj": np.ascontiguousarray(
                W_proj[lo:lo + FC, :].astype(ml_dtypes.bfloat16)),
        })
    return in_maps


def kernel(x, W_attn, b_attn, W_proj, b_proj, **run_kwargs):
    if "nc" not in _CACHE:
        _CACHE["nc"] = build_program()
    nc = _CACHE["nc"]
    in_maps = make_in_maps(x, W_attn, b_attn, W_proj)
    res = run_bass_kernel_spmd(nc, in_maps, core_ids=list(range(NCORES)), **run_kwargs)
    _CACHE["last_results"] = res
    total = np.zeros((NT, C), dtype=np.float32)
    for r in res.results:
        total += np.asarray(r["out"], dtype=np.float32)
    total += np.asarray(b_proj, dtype=np.float32)[None, :]
    return total.reshape(B, T, C)


# revision 34
# speedup vs baseline: 1.0607x; 1.0287x over previous
"""Causal self-attention (B=4, T=2048, C=1024, H=16) on 8 TRN2 NeuronCores.

Sharding: tensor-parallel over heads — 2 heads per core. Each core gets the
full x (replicated, bf16 + fp8 copies), its W_attn column slice (q|k|v for
its 2 heads), and its 128-row slice of W_proj; it produces a full-shape
[B*T, C] fp16 partial output which the host sums across cores (b_proj added
on host).

Per-core pipeline (activations feature-on-partition, "transposed"):
  A. x^T loads (host-pretransposed): fp8 for the K/Q projections, bf16
     for V.  Later batches prefetch during the previous batch's compute.
  B. qkv^T[f, t] = W.T @ x^T (+bias); K and Q via fp8 DoubleRow
     (contraction 256 per pass, W upscaled x64 on host so e4m3 stays in
     normal range).  V^T -> vaug = [V_h | 1] slots via one PE transpose
     plus one strided DVE copy per kt chunk.
  C. Per (j, kc), diagonal chunks first (scores before any PV so the PE
     queue never head-of-line blocks on the psy bank handoff): both
     heads' score matmuls pack one [128, 2*512] PSUM tile (row-band
     tile_position); ONE exp ACT over the causally-needed column suffix
     only; causal triangle masked in-place by GpSimd affine_select over
     the first 128 suffix columns (fill=0); per-head [V|1].T @ P^T
     accumulated into one [128, 2, 512] PSUM tile (65 live rows per
     head; row 64 = softmax denominator).
     Normalization is latency-split: an ACT copy (y rows) plus a DVE
     copy (denom row -> partition 0) release the psy banks immediately;
     the reciprocal, GpSimd partition broadcast and the multiplies are
     deferred to the NEXT j boundary so the GpSimd queue never stalls
     ahead of the next j's causal selects.
  D. out = y^T.T @ W_proj per 128-token chunk, drained to fp16 and
     DMA'd out two chunks per descriptor.  Emission is software-
     pipelined: phase D chunks and the NEXT batch's projection groups
     are interleaved at the phase-C j boundaries so the PE stays fed
     while each j's normalize chain completes.

PSUM budget (8 banks): ps_s 2x[128, 2*512] score tiles; ps_yo
1x[128, 2, 512] PV accumulator; ps_io 2x[128, 512] shared by phase-B
projection tiles, V-transposes and phase-D output tiles in strict
emission alternation.
"""

import sys
import numpy as np

if "/opt/trn_rl_repo" not in sys.path:
    sys.path.insert(0, "/opt/trn_rl_repo")

from contextlib import ExitStack

import ml_dtypes
import concourse.bass as bass
import concourse.mybir as mybir
import concourse.tile as tile
from concourse import bacc
from concourse.bass_utils import run_bass_kernel_spmd
from concourse.masks import make_identity

B, T, C, H, D = 4, 2048, 1024, 16, 64
P = 128
NCORES = 8
HPC = H // NCORES          # 2 heads per core
FC = HPC * D               # 128 features per core per q/k/v
NT = B * T                 # 8192 tokens
CC = C // P                # 8 contraction chunks
TJ = 512                   # token tile (free dim) for big matmuls
NQ = T // TJ               # 4 qt chunks per batch
KCH = T // P               # 16 kt chunks per batch
F32 = mybir.dt.float32
F16 = mybir.dt.float16
BF16 = mybir.dt.bfloat16
FP8 = mybir.dt.float8e4
AF = mybir.ActivationFunctionType
ALU = mybir.AluOpType
DR = mybir.MatmulPerfMode.DoubleRow
W_SCALE = 64.0  # host-side upscale of fp8 W slices so e4m3 stays normal-range
Q_FP8 = True   # Q projection via fp8 DoubleRow (cheaper, rel_err ~1.8e-2)

_CACHE = {}


def build_program():
    nc = bacc.Bacc("TRN2", target_bir_lowering=False, debug=False)

    xt_d = nc.dram_tensor("xt", [C, NT], BF16, kind="ExternalInput").ap()
    xt8_d = nc.dram_tensor("xt8", [C, NT], FP8, kind="ExternalInput").ap()
    nqv = FC if Q_FP8 else 2 * FC
    wa_d = nc.dram_tensor("w_qv", [C, nqv], BF16, kind="ExternalInput").ap()
    n8 = 2 * FC if Q_FP8 else FC
    w8_d = nc.dram_tensor("w_k8", [C, n8], FP8, kind="ExternalInput").ap()
    ba_d = nc.dram_tensor("b_attn", [3, FC], F32, kind="ExternalInput").ap()
    wp_d = nc.dram_tensor("w_proj", [FC, C], BF16, kind="ExternalInput").ap()
    out_d = nc.dram_tensor("out", [NT, C], F16, kind="ExternalOutput").ap()

    with tile.TileContext(nc) as tc, ExitStack() as ctx:
        consts = ctx.enter_context(tc.tile_pool(name="consts", bufs=1))
        xt_pool = ctx.enter_context(tc.tile_pool(name="xt", bufs=2))
        qkvt_pool = ctx.enter_context(tc.tile_pool(name="qkvt", bufs=2))
        vaug_pool = ctx.enter_context(tc.tile_pool(name="vaug", bufs=2))
        pt_pool = ctx.enter_context(tc.tile_pool(name="pt", bufs=6))
        sums_pool = ctx.enter_context(tc.tile_pool(name="sums", bufs=2))
        yu_pool = ctx.enter_context(tc.tile_pool(name="yu", bufs=3))
        rbc_pool = ctx.enter_context(tc.tile_pool(name="rbc", bufs=2))
        y_pool = ctx.enter_context(tc.tile_pool(name="y", bufs=8))
        o_pool = ctx.enter_context(tc.tile_pool(name="o", bufs=3))

        ps_s = ctx.enter_context(tc.tile_pool(name="ps_s", bufs=2, space="PSUM"))
        ps_yo = ctx.enter_context(tc.tile_pool(name="ps_yo", bufs=1, space="PSUM"))
        ps_io = ctx.enter_context(tc.tile_pool(name="ps_io", bufs=2, space="PSUM"))

        def io_tile(shape, name):
            # phase-B projection tiles and phase-D output tiles cycle the
            # same two 1-bank ps_io buffers in strict emission alternation.
            return ps_io.tile(shape, F32, name=name, tag="ps_io")

        # --- constants needed by the first projection groups ---
        wa_r = wa_d.rearrange("(cc p) f -> p cc f", p=P)
        w8_r = w8_d.rearrange("(cc p) f -> p cc f", p=P)
        w8_sb = consts.tile([P, CC, n8], FP8)
        nc.sync.dma_start(w8_sb[:], w8_r)
        w_qv_sb = consts.tile([P, CC, nqv], BF16)
        nc.sync.dma_start(w_qv_sb[:], wa_r)
        bias_sb = consts.tile([P, 3], F32)
        nc.scalar.dma_start(bias_sb[:], ba_d.rearrange("f p -> p f"))
        # --- later-needed constants, emitted after batch 0's x loads ---
        wp_sb = consts.tile([P, C], BF16)
        ident = consts.tile([P, P], F32)
        identb = consts.tile([P, P], BF16)
        ones_st = consts.tile([P, 1], F32)
        ones_b = consts.tile([P, 1], BF16)

        def emit_late_consts():
            nc.sync.dma_start(wp_sb[:], wp_d)
            make_identity(nc, ident[:])
            nc.vector.tensor_copy(out=identb[:], in_=ident[:])
            nc.vector.memset(ones_st[:], 1.0)
            nc.vector.tensor_copy(out=ones_b[:], in_=ones_st[:])

        # Phase D of batch b is deferred and emitted interleaved with phase
        # B of batch b+1.  Each pending entry is one 128-token chunk; chunk
        # pairs share an ost tile so the output DMA moves 256 rows at once.
        pending_d = []

        def emit_d_chunk(j0row, ysb, tb, ost, split_drain=False):
            par = tb % 2
            ts = slice(tb * P, (tb + 1) * P)
            for cn in range(C // TJ):
                pso = io_tile([P, TJ], "pso")
                nc.tensor.matmul(
                    pso[:],
                    ysb[:, ts],
                    wp_sb[:, cn * TJ:(cn + 1) * TJ],
                    start=True,
                    stop=True,
                )
                osl = ost[:, par, cn * TJ:(cn + 1) * TJ]
                if split_drain and cn == 0:
                    nc.scalar.copy(osl, pso[:])
                else:
                    nc.vector.tensor_copy(out=osl, in_=pso[:])
            if par == 1:
                r0 = j0row + (tb - 1) * P
                nc.sync.dma_start(
                    out_d[r0:r0 + 2 * P, :].rearrange("(two p) c -> p two c", p=P),
                    ost[:],
                )

        def emit_d_pair(j0row, ysb, pair, split_drain=False):
            ost = o_pool.tile([P, 2, C], F16, name="ost", tag="ost")
            emit_d_chunk(j0row, ysb, 2 * pair, ost, split_drain)
            emit_d_chunk(j0row, ysb, 2 * pair + 1, ost, split_drain)

        xt8_r = xt8_d.rearrange("(cc p) t -> p cc t", p=P)

        def alloc_x_tiles():
            x8 = [xt_pool.tile([P, 2, T], FP8, name=f"xt8_{c2}",
                               tag=f"xt8_{c2}") for c2 in range(CC // 2)]
            xb = [xt_pool.tile([P, T], BF16, name=f"xt{cc}",
                               tag=f"xt{cc}") for cc in range(CC)]
            return xb, x8

        def emit_prefetch(bn, tiles):
            xb, x8 = tiles
            nt0 = bn * T
            for c2 in range(CC // 2):
                nc.sync.dma_start(x8[c2][:],
                                  xt8_r[:, 2 * c2:2 * c2 + 2, nt0:nt0 + T])
            for cc in range(CC):
                nc.sync.dma_start(xb[cc][:],
                                  xt_d[cc * P:(cc + 1) * P, nt0:nt0 + T])

        def emit_proj_group(tj, f, xtiles, qkvt, vaug):
            # one (tj, f) projection group; after V (f==2), transpose the
            # tj's four V chunks into vaug
            xtb, xt8b = xtiles
            tjs = slice(tj * TJ, (tj + 1) * TJ)
            psf = io_tile([P, TJ], "psf")
            if f in ((1, 0) if Q_FP8 else (1,)):
                w8o = 0 if f == 1 else FC
                for c2 in range(CC // 2):
                    nc.tensor.matmul(
                        psf[:],
                        w8_sb[:, 2 * c2:2 * c2 + 2, w8o:w8o + FC],
                        xt8b[c2][:, :, tjs],
                        start=(c2 == 0),
                        stop=(c2 == CC // 2 - 1),
                        perf_mode=DR,
                    )
                nc.vector.tensor_scalar(
                    out=qkvt[:, f, tjs], in0=psf[:],
                    scalar1=float(1.0 / W_SCALE),
                    scalar2=bias_sb[:, f:f + 1],
                    op0=ALU.mult, op1=ALU.add,
                )
            else:
                wo = 0 if (f == 0 and not Q_FP8) else nqv - FC
                for cc in range(CC):
                    nc.tensor.matmul(
                        psf[:],
                        w_qv_sb[:, cc, wo:wo + FC],
                        xtb[cc][:, tjs],
                        start=(cc == 0),
                        stop=(cc == CC - 1),
                    )
                nc.vector.tensor_scalar_add(
                    qkvt[:, f, tjs], psf[:], bias_sb[:, f:f + 1]
                )
            if f == 2:
                for kc in range(4 * tj, 4 * tj + 4):
                    pst = ps_io.tile([P, P], BF16, name="pst", tag="ps_io")
                    nc.tensor.transpose(
                        pst[:], qkvt[:, 2, kc * P:(kc + 1) * P], identb[:])
                    nc.vector.tensor_copy(
                        out=vaug[:, kc, :, 0:D], in_=pst[:])

        # ---- batch 0: phase A + full phase B inline ----
        x_tiles = [None] * (B + 1)
        qkvts = [None] * B
        vaugs = [None] * B
        x_tiles[0] = alloc_x_tiles()
        xtb0, xt8b0 = x_tiles[0]
        for half in range(2):
            hs = slice(half * (T // 2), (half + 1) * (T // 2))
            for c2 in range(CC // 2):
                q = nc.scalar if c2 % 2 else nc.sync
                q.dma_start(xt8b0[c2][:, :, hs], xt8_r[:, 2 * c2:2 * c2 + 2, hs])
            for cc in range(CC):
                q = nc.scalar if cc % 2 else nc.sync
                q.dma_start(xtb0[cc][:, hs], xt_d[cc * P:(cc + 1) * P, hs])
            if half == 0:
                emit_late_consts()
        del xtb0, xt8b0
        qkvts[0] = qkvt_pool.tile([P, 3, T], BF16, name="qkvt")
        vaugs[0] = vaug_pool.tile([P, KCH, HPC, D + 1], BF16, name="vaug")
        nc.vector.tensor_copy(
            out=vaugs[0][:, :, :, D:D + 1],
            in_=ones_b[:, None, None, :].to_broadcast((P, KCH, HPC, 1)),
        )
        for tj in range(NQ):
            for f in (1, 0, 2):
                emit_proj_group(tj, f, x_tiles[0], qkvts[0], vaugs[0])
        x_tiles[1] = alloc_x_tiles()
        emit_prefetch(1, x_tiles[1])

        # pending output chunks: (row0, ysb, pair) emitted at later j
        # boundaries so their matmuls never head-of-line block the PE
        pending_d = []
        # deferred second half of the softmax normalization
        pending_norm = []

        def emit_norm_finish(yu, recip, row0):
            rbc = rbc_pool.tile([D, HPC, TJ], F32, name="rbc", tag="rbc")
            nc.gpsimd.partition_broadcast(rbc[:], recip[:])
            ysb = y_pool.tile([P, TJ], BF16, name="ysb")
            for h in range(HPC):
                nc.vector.tensor_mul(
                    out=ysb[h * D:(h + 1) * D, :],
                    in0=yu[:D, h, :],
                    in1=rbc[:, h, :],
                )
            for pair in range(TJ // P // 2):
                pending_d.append((row0, ysb, pair))
        PACE = (2, 3, 3, 4)   # B(b+1) proj groups per j boundary
        PACE0 = (0, 4, 4, 4)  # batch 0: defer past the tight first boundary

        for b in range(B):
            t0 = b * T
            qkvt = qkvts[b]
            vaug = vaugs[b]
            b_work = []
            if b + 1 < B:
                qkvts[b + 1] = qkvt_pool.tile([P, 3, T], BF16, name="qkvt")
                vaugs[b + 1] = vaug_pool.tile([P, KCH, HPC, D + 1], BF16, name="vaug")
                nc.vector.tensor_copy(
                    out=vaugs[b + 1][:, :, :, D:D + 1],
                    in_=ones_b[:, None, None, :].to_broadcast((P, KCH, HPC, 1)),
                )
                b_work = [(tj, f) for tj in range(NQ) for f in (1, 0, 2)]

            # ---- phase C ----
            for j in range(NQ):
                nkc = 4 * j + 4
                psy = ps_yo.tile([P, HPC, TJ], F32, name="psy", tag="psy")
                kc_order = [4 * j + 3, 4 * j + 2, 4 * j + 1, 4 * j] + \
                    list(range(0, 4 * j))

                def emit_scores(kc):
                    # scores + exp (+ causal select) over the causally
                    # needed column suffix; returns (pt tile, suffix)
                    r = kc - 4 * j
                    cs = slice(r * P, TJ) if r > 0 else slice(0, TJ)
                    pss = ps_s.tile([P, HPC, TJ], F32, name="pss", tag="pss")
                    for h in range(HPC):
                        hd = slice(h * D, (h + 1) * D)
                        nc.tensor.matmul(
                            pss[:, h, cs],
                            qkvt[hd, 1, kc * P:(kc + 1) * P],
                            qkvt[hd, 0, j * TJ + cs.start:(j + 1) * TJ],
                            start=True,
                            stop=True,
                            tile_position=(h * D, 0),
                        )
                    pt = pt_pool.tile([P, HPC, TJ], BF16, name="pt", tag="pt")
                    nc.scalar.activation(
                        pt[:, :, cs], pss[:, :, cs], AF.Exp,
                        bias=0.0, scale=float(1.0 / np.sqrt(D)),
                    )
                    if r >= 0:
                        # keep pt[kt, h, q] only where q >= kt (both heads);
                        # only the first 128 suffix columns can be masked
                        # (kt < 128), so the select stops there
                        ms = slice(cs.start, cs.start + P)
                        nc.gpsimd.affine_select(
                            out=pt[:, :, ms],
                            in_=pt[:, :, ms],
                            compare_op=ALU.is_ge,
                            fill=0.0,
                            base=0,
                            pattern=[[0, HPC], [1, P]],
                            channel_multiplier=-1,
                        )
                    return pt, cs

                def emit_pv(kc, pt, cs, first, last):
                    for h in range(HPC):
                        nc.tensor.matmul(
                            psy[:D + 1, h, cs],
                            vaug[:, kc, h, :],
                            pt[:, h, cs],
                            start=first,
                            stop=last,
                        )

                # diagonal chunks: all scores first (their PVs are the
                # first psy writers and may briefly wait on the previous
                # j's psy release — keep the PE queue fed with scores)
                diag = kc_order[:4]
                diag_pt = [emit_scores(kc) for kc in diag]
                for i, kc in enumerate(diag):
                    pt, cs = diag_pt[i]
                    emit_pv(kc, pt, cs, i == 0, i == nkc - 1)
                for i, kc in enumerate(kc_order[4:]):
                    pt, cs = emit_scores(kc)
                    emit_pv(kc, pt, cs, False, i == nkc - 5)

                # normalization, first half: drain psy fast on two
                # parallel engines (body rows -> yu on ACT, denom row 64 ->
                # sums on DVE) so the next j's PV can take the banks; the
                # partition broadcast and the multiplies are deferred one
                # boundary so the GpSimd queue never waits ahead of the
                # next j's causal selects.  sums lands on partition 0 — the
                # custom-DVE reciprocal mishandles offset partitions.
                yu = yu_pool.tile([D, HPC, TJ], F32, name="yu", tag="yu")
                nc.scalar.copy(yu[:], psy[:D, :, :])
                sums = sums_pool.tile([1, HPC, TJ], F32, name="sums",
                                      tag="sums")
                nc.vector.tensor_copy(out=sums[:], in_=psy[D:D + 1, :, :])
                recip = sums_pool.tile([1, HPC, TJ], F32, name="recip",
                                       tag="recip")
                nc.vector.reciprocal_approx_fast(out=recip[:], in_=sums[:])

                # ---- j-boundary fill work (keeps PE fed while the
                #      normalize chain for this j completes) ----
                for _ in range(4 if b == B - 1 else 2):
                    if pending_d:
                        emit_d_pair(*pending_d.pop(0))
                if pending_norm:
                    emit_norm_finish(*pending_norm.pop(0))
                pending_norm.append((yu, recip, t0 + j * TJ))
                for _ in range((PACE0 if b == 0 else PACE)[j]):
                    if b_work:
                        tj, f = b_work.pop(0)
                        emit_proj_group(tj, f, x_tiles[b + 1],
                                        qkvts[b + 1], vaugs[b + 1])

            while b_work:
                tj, f = b_work.pop(0)
                emit_proj_group(tj, f, x_tiles[b + 1], qkvts[b + 1],
                                vaugs[b + 1])
            if b + 2 < B:
                x_tiles[b + 2] = alloc_x_tiles()
                emit_prefetch(b + 2, x_tiles[b + 2])

        # tail: remaining normalize + output chunks of the last batch
        while pending_norm:
            emit_norm_finish(*pending_norm.pop(0))
        while pending_d:
            emit_d_pair(*pending_d.pop(0), split_drain=True)

    nc.compile()
    return nc


def make_in_maps(x, W_attn, b_attn, W_proj):
    x_flat = np.asarray(x, dtype=np.float32).reshape(NT, C)
    xt = np.ascontiguousarray(x_flat.T)
    xt_bf = xt.astype(ml_dtypes.bfloat16)
    xt_f8 = xt.astype(ml_dtypes.float8_e4m3)
    W_attn = np.asarray(W_attn, dtype=np.float32)
    b_attn = np.asarray(b_attn, dtype=np.float32)
    W_proj = np.asarray(W_proj, dtype=np.float32)
    in_maps = []
    for core in range(NCORES):
        lo = core * FC
        cols = np.concatenate(
            [np.arange(lo, lo + FC) + k * C for k in range(3)]
        )
        w_slice = W_attn[:, cols]
        wq, wk, wv = w_slice[:, :FC], w_slice[:, FC:2 * FC], w_slice[:, 2 * FC:]
        if Q_FP8:
            w_qv = wv
            w_k8 = np.concatenate([wk, wq], axis=1)
        else:
            w_qv = np.concatenate([wq, wv], axis=1)
            w_k8 = wk
        in_maps.append({
            "xt": xt_bf,
            "xt8": xt_f8,
            "w_qv": np.ascontiguousarray(w_qv.astype(ml_dtypes.bfloat16)),
            "w_k8": np.ascontiguousarray(
                (w_k8 * W_SCALE).astype(ml_dtypes.float8_e4m3)),
            "b_attn": np.ascontiguousarray(b_attn[cols].reshape(3, FC)),
            "w_proj": np.ascontiguousarray(
                W_proj[lo:lo + FC, :].astype(ml_dtypes.bfloat16)),
        })
    return in_maps


def kernel(x, W_attn, b_attn, W_proj, b_proj, **run_kwargs):
    if "nc" not in _CACHE:
        _CACHE["nc"] = build_program()
    nc = _CACHE["nc"]
    in_maps = make_in_maps(x, W_attn, b_attn, W_proj)
    res = run_bass_kernel_spmd(nc, in_maps, core_ids=list(range(NCORES)), **run_kwargs)
    _CACHE["last_results"] = res
    total = np.zeros((NT, C), dtype=np.float32)
    for r in res.results:
        total += np.asarray(r["out"], dtype=np.float32)
    total += np.asarray(b_proj, dtype=np.float32)[None, :]
    return total.reshape(B, T, C)


# revision 35
# speedup vs baseline: 1.0637x; 1.0028x over previous
"""Causal self-attention (B=4, T=2048, C=1024, H=16) on 8 TRN2 NeuronCores.

Sharding: tensor-parallel over heads — 2 heads per core. Each core gets the
full x (replicated, bf16 + fp8 copies), its W_attn column slice (q|k|v for
its 2 heads), and its 128-row slice of W_proj; it produces a full-shape
[B*T, C] fp16 partial output which the host sums across cores (b_proj added
on host).

Per-core pipeline (activations feature-on-partition, "transposed"):
  A. x^T loads (host-pretransposed): fp8 for the K/Q projections, bf16
     for V.  Later batches prefetch during the previous batch's compute.
  B. qkv^T[f, t] = W.T @ x^T (+bias); K and Q via fp8 DoubleRow
     (contraction 256 per pass, W upscaled x64 on host so e4m3 stays in
     normal range).  V^T -> vaug = [V_h | 1] slots via one PE transpose
     plus one strided DVE copy per kt chunk.
  C. Per (j, kc), diagonal chunks first (scores before any PV so the PE
     queue never head-of-line blocks on the psy bank handoff): both
     heads' score matmuls pack one [128, 2*512] PSUM tile (row-band
     tile_position); ONE exp ACT over the causally-needed column suffix
     only; causal triangle masked in-place by GpSimd affine_select over
     the first 128 suffix columns (fill=0); per-head [V|1].T @ P^T
     accumulated into one [128, 2, 512] PSUM tile (65 live rows per
     head; row 64 = softmax denominator).
     Normalization is latency-split: an ACT copy (y rows) plus a DVE
     copy (denom row -> partition 0) release the psy banks immediately;
     the reciprocal, GpSimd partition broadcast and the multiplies are
     deferred to the NEXT j boundary so the GpSimd queue never stalls
     ahead of the next j's causal selects.
  D. out = y^T.T @ W_proj per 128-token chunk, drained to fp16 and
     DMA'd out two chunks per descriptor.  Emission is software-
     pipelined: phase D chunks and the NEXT batch's projection groups
     are interleaved at the phase-C j boundaries so the PE stays fed
     while each j's normalize chain completes.

PSUM budget (8 banks): ps_s 2x[128, 2*512] score tiles; ps_yo
1x[128, 2, 512] PV accumulator; ps_io 2x[128, 512] shared by phase-B
projection tiles, V-transposes and phase-D output tiles in strict
emission alternation.
"""

import sys
import numpy as np

if "/opt/trn_rl_repo" not in sys.path:
    sys.path.insert(0, "/opt/trn_rl_repo")

from contextlib import ExitStack

import ml_dtypes
import concourse.bass as bass
import concourse.mybir as mybir
import concourse.tile as tile
from concourse import bacc
from concourse.bass_utils import run_bass_kernel_spmd
from concourse.masks import make_identity

B, T, C, H, D = 4, 2048, 1024, 16, 64
P = 128
NCORES = 8
HPC = H // NCORES          # 2 heads per core
FC = HPC * D               # 128 features per core per q/k/v
NT = B * T                 # 8192 tokens
CC = C // P                # 8 contraction chunks
TJ = 512                   # token tile (free dim) for big matmuls
NQ = T // TJ               # 4 qt chunks per batch
KCH = T // P               # 16 kt chunks per batch
F32 = mybir.dt.float32
F16 = mybir.dt.float16
BF16 = mybir.dt.bfloat16
FP8 = mybir.dt.float8e4
AF = mybir.ActivationFunctionType
ALU = mybir.AluOpType
DR = mybir.MatmulPerfMode.DoubleRow
W_SCALE = 64.0  # host-side upscale of fp8 W slices so e4m3 stays normal-range
Q_FP8 = True   # Q projection via fp8 DoubleRow (cheaper, rel_err ~1.8e-2)

_CACHE = {}


def build_program():
    nc = bacc.Bacc("TRN2", target_bir_lowering=False, debug=False)

    xt_d = nc.dram_tensor("xt", [C, NT], BF16, kind="ExternalInput").ap()
    xt8_d = nc.dram_tensor("xt8", [C, NT], FP8, kind="ExternalInput").ap()
    nqv = FC if Q_FP8 else 2 * FC
    wa_d = nc.dram_tensor("w_qv", [C, nqv], BF16, kind="ExternalInput").ap()
    n8 = 2 * FC if Q_FP8 else FC
    w8_d = nc.dram_tensor("w_k8", [C, n8], FP8, kind="ExternalInput").ap()
    ba_d = nc.dram_tensor("b_attn", [3, FC], F32, kind="ExternalInput").ap()
    wp_d = nc.dram_tensor("w_proj", [FC, C], BF16, kind="ExternalInput").ap()
    out_d = nc.dram_tensor("out", [NT, C], F16, kind="ExternalOutput").ap()

    with tile.TileContext(nc) as tc, ExitStack() as ctx:
        consts = ctx.enter_context(tc.tile_pool(name="consts", bufs=1))
        xt_pool = ctx.enter_context(tc.tile_pool(name="xt", bufs=2))
        qkvt_pool = ctx.enter_context(tc.tile_pool(name="qkvt", bufs=2))
        vaug_pool = ctx.enter_context(tc.tile_pool(name="vaug", bufs=2))
        pt_pool = ctx.enter_context(tc.tile_pool(name="pt", bufs=6))
        sums_pool = ctx.enter_context(tc.tile_pool(name="sums", bufs=2))
        yu_pool = ctx.enter_context(tc.tile_pool(name="yu", bufs=3))
        rbc_pool = ctx.enter_context(tc.tile_pool(name="rbc", bufs=2))
        y_pool = ctx.enter_context(tc.tile_pool(name="y", bufs=8))
        o_pool = ctx.enter_context(tc.tile_pool(name="o", bufs=3))

        ps_s = ctx.enter_context(tc.tile_pool(name="ps_s", bufs=2, space="PSUM"))
        ps_yo = ctx.enter_context(tc.tile_pool(name="ps_yo", bufs=1, space="PSUM"))
        ps_io = ctx.enter_context(tc.tile_pool(name="ps_io", bufs=2, space="PSUM"))

        def io_tile(shape, name):
            # phase-B projection tiles and phase-D output tiles cycle the
            # same two 1-bank ps_io buffers in strict emission alternation.
            return ps_io.tile(shape, F32, name=name, tag="ps_io")

        # --- constants needed by the first projection groups ---
        wa_r = wa_d.rearrange("(cc p) f -> p cc f", p=P)
        w8_r = w8_d.rearrange("(cc p) f -> p cc f", p=P)
        w8_sb = consts.tile([P, CC, n8], FP8)
        nc.sync.dma_start(w8_sb[:], w8_r)
        w_qv_sb = consts.tile([P, CC, nqv], BF16)
        nc.sync.dma_start(w_qv_sb[:], wa_r)
        bias_sb = consts.tile([P, 3], F32)
        nc.scalar.dma_start(bias_sb[:], ba_d.rearrange("f p -> p f"))
        # --- later-needed constants, emitted after batch 0's x loads ---
        wp_sb = consts.tile([P, C], BF16)
        ident = consts.tile([P, P], F32)
        identb = consts.tile([P, P], BF16)
        ones_st = consts.tile([P, 1], F32)
        ones_b = consts.tile([P, 1], BF16)

        def emit_late_consts():
            nc.sync.dma_start(wp_sb[:], wp_d)
            make_identity(nc, ident[:])
            nc.vector.tensor_copy(out=identb[:], in_=ident[:])
            nc.vector.memset(ones_st[:], 1.0)
            nc.vector.tensor_copy(out=ones_b[:], in_=ones_st[:])

        # Phase D of batch b is deferred and emitted interleaved with phase
        # B of batch b+1.  Each pending entry is one 128-token chunk; chunk
        # pairs share an ost tile so the output DMA moves 256 rows at once.
        pending_d = []

        def emit_d_chunk(j0row, ysb, tb, ost, split_drain=False):
            par = tb % 2
            ts = slice(tb * P, (tb + 1) * P)
            for cn in range(C // TJ):
                pso = io_tile([P, TJ], "pso")
                nc.tensor.matmul(
                    pso[:],
                    ysb[:, ts],
                    wp_sb[:, cn * TJ:(cn + 1) * TJ],
                    start=True,
                    stop=True,
                )
                osl = ost[:, par, cn * TJ:(cn + 1) * TJ]
                if split_drain and cn == 0:
                    nc.scalar.copy(osl, pso[:])
                else:
                    nc.vector.tensor_copy(out=osl, in_=pso[:])
            if par == 1:
                r0 = j0row + (tb - 1) * P
                nc.sync.dma_start(
                    out_d[r0:r0 + 2 * P, :].rearrange("(two p) c -> p two c", p=P),
                    ost[:],
                )

        def emit_d_pair(j0row, ysb, pair, split_drain=False):
            ost = o_pool.tile([P, 2, C], F16, name="ost", tag="ost")
            emit_d_chunk(j0row, ysb, 2 * pair, ost, split_drain)
            emit_d_chunk(j0row, ysb, 2 * pair + 1, ost, split_drain)

        xt8_r = xt8_d.rearrange("(cc p) t -> p cc t", p=P)

        def alloc_x_tiles():
            x8 = [xt_pool.tile([P, 2, T], FP8, name=f"xt8_{c2}",
                               tag=f"xt8_{c2}") for c2 in range(CC // 2)]
            xb = [xt_pool.tile([P, T], BF16, name=f"xt{cc}",
                               tag=f"xt{cc}") for cc in range(CC)]
            return xb, x8

        def emit_prefetch(bn, tiles):
            xb, x8 = tiles
            nt0 = bn * T
            for c2 in range(CC // 2):
                nc.sync.dma_start(x8[c2][:],
                                  xt8_r[:, 2 * c2:2 * c2 + 2, nt0:nt0 + T])
            for cc in range(CC):
                nc.sync.dma_start(xb[cc][:],
                                  xt_d[cc * P:(cc + 1) * P, nt0:nt0 + T])

        def emit_proj_group(tj, f, xtiles, qkvt, vaug):
            # one (tj, f) projection group; after V (f==2), transpose the
            # tj's four V chunks into vaug
            xtb, xt8b = xtiles
            tjs = slice(tj * TJ, (tj + 1) * TJ)
            psf = io_tile([P, TJ], "psf")
            if f in ((1, 0) if Q_FP8 else (1,)):
                w8o = 0 if f == 1 else FC
                for c2 in range(CC // 2):
                    nc.tensor.matmul(
                        psf[:],
                        w8_sb[:, 2 * c2:2 * c2 + 2, w8o:w8o + FC],
                        xt8b[c2][:, :, tjs],
                        start=(c2 == 0),
                        stop=(c2 == CC // 2 - 1),
                        perf_mode=DR,
                    )
                nc.vector.tensor_scalar(
                    out=qkvt[:, f, tjs], in0=psf[:],
                    scalar1=float(1.0 / W_SCALE),
                    scalar2=bias_sb[:, f:f + 1],
                    op0=ALU.mult, op1=ALU.add,
                )
            else:
                wo = 0 if (f == 0 and not Q_FP8) else nqv - FC
                for cc in range(CC):
                    nc.tensor.matmul(
                        psf[:],
                        w_qv_sb[:, cc, wo:wo + FC],
                        xtb[cc][:, tjs],
                        start=(cc == 0),
                        stop=(cc == CC - 1),
                    )
                nc.vector.tensor_scalar_add(
                    qkvt[:, f, tjs], psf[:], bias_sb[:, f:f + 1]
                )
            if f == 2:
                for kc in range(4 * tj, 4 * tj + 4):
                    pst = ps_io.tile([P, P], BF16, name="pst", tag="ps_io")
                    nc.tensor.transpose(
                        pst[:], qkvt[:, 2, kc * P:(kc + 1) * P], identb[:])
                    nc.vector.tensor_copy(
                        out=vaug[:, kc, :, 0:D], in_=pst[:])

        # ---- batch 0: phase A + full phase B inline ----
        x_tiles = [None] * (B + 1)
        qkvts = [None] * B
        vaugs = [None] * B
        x_tiles[0] = alloc_x_tiles()
        xtb0, xt8b0 = x_tiles[0]
        for half in range(2):
            hs = slice(half * (T // 2), (half + 1) * (T // 2))
            for c2 in range(CC // 2):
                q = nc.scalar if c2 % 2 else nc.sync
                q.dma_start(xt8b0[c2][:, :, hs], xt8_r[:, 2 * c2:2 * c2 + 2, hs])
            for cc in range(CC):
                q = nc.scalar if cc % 2 else nc.sync
                q.dma_start(xtb0[cc][:, hs], xt_d[cc * P:(cc + 1) * P, hs])
            if half == 0:
                emit_late_consts()
        del xtb0, xt8b0
        qkvts[0] = qkvt_pool.tile([P, 3, T], BF16, name="qkvt")
        vaugs[0] = vaug_pool.tile([P, KCH, HPC, D + 1], BF16, name="vaug")
        nc.vector.tensor_copy(
            out=vaugs[0][:, :, :, D:D + 1],
            in_=ones_b[:, None, None, :].to_broadcast((P, KCH, HPC, 1)),
        )
        for tj in range(NQ):
            for f in (1, 0, 2):
                emit_proj_group(tj, f, x_tiles[0], qkvts[0], vaugs[0])
        x_tiles[1] = alloc_x_tiles()
        emit_prefetch(1, x_tiles[1])

        # pending output chunks: (row0, ysb, pair) emitted at later j
        # boundaries so their matmuls never head-of-line block the PE
        pending_d = []
        # deferred second half of the softmax normalization
        pending_norm = []

        def emit_norm_finish(yu, recip, row0):
            rbc = rbc_pool.tile([D, HPC, TJ], F32, name="rbc", tag="rbc")
            nc.gpsimd.partition_broadcast(rbc[:], recip[:])
            ysb = y_pool.tile([P, TJ], BF16, name="ysb")
            for h in range(HPC):
                nc.vector.tensor_mul(
                    out=ysb[h * D:(h + 1) * D, :],
                    in0=yu[:D, h, :],
                    in1=rbc[:, h, :],
                )
            for pair in range(TJ // P // 2):
                pending_d.append((row0, ysb, pair))
        PACE = (2, 3, 3, 4)   # B(b+1) proj groups per j boundary
        PACE0 = (0, 4, 4, 4)  # batch 0: defer past the tight first boundary

        for b in range(B):
            t0 = b * T
            qkvt = qkvts[b]
            vaug = vaugs[b]
            b_work = []
            if b + 1 < B:
                qkvts[b + 1] = qkvt_pool.tile([P, 3, T], BF16, name="qkvt")
                vaugs[b + 1] = vaug_pool.tile([P, KCH, HPC, D + 1], BF16, name="vaug")
                nc.vector.tensor_copy(
                    out=vaugs[b + 1][:, :, :, D:D + 1],
                    in_=ones_b[:, None, None, :].to_broadcast((P, KCH, HPC, 1)),
                )
                b_work = [(tj, f) for tj in range(NQ) for f in (1, 0, 2)]

            # ---- phase C ----
            fill_q = []
            for j in range(NQ):
                nkc = 4 * j + 4
                psy = ps_yo.tile([P, HPC, TJ], F32, name="psy", tag="psy")
                kc_order = [4 * j + 3, 4 * j + 2, 4 * j + 1, 4 * j] + \
                    list(range(0, 4 * j))

                def emit_scores(kc):
                    # scores + exp (+ causal select) over the causally
                    # needed column suffix; returns (pt tile, suffix)
                    r = kc - 4 * j
                    cs = slice(r * P, TJ) if r > 0 else slice(0, TJ)
                    pss = ps_s.tile([P, HPC, TJ], F32, name="pss", tag="pss")
                    for h in range(HPC):
                        hd = slice(h * D, (h + 1) * D)
                        nc.tensor.matmul(
                            pss[:, h, cs],
                            qkvt[hd, 1, kc * P:(kc + 1) * P],
                            qkvt[hd, 0, j * TJ + cs.start:(j + 1) * TJ],
                            start=True,
                            stop=True,
                            tile_position=(h * D, 0),
                        )
                    pt = pt_pool.tile([P, HPC, TJ], BF16, name="pt", tag="pt")
                    nc.scalar.activation(
                        pt[:, :, cs], pss[:, :, cs], AF.Exp,
                        bias=0.0, scale=float(1.0 / np.sqrt(D)),
                    )
                    if r >= 0:
                        # keep pt[kt, h, q] only where q >= kt (both heads);
                        # only the first 128 suffix columns can be masked
                        # (kt < 128), so the select stops there
                        ms = slice(cs.start, cs.start + P)
                        nc.gpsimd.affine_select(
                            out=pt[:, :, ms],
                            in_=pt[:, :, ms],
                            compare_op=ALU.is_ge,
                            fill=0.0,
                            base=0,
                            pattern=[[0, HPC], [1, P]],
                            channel_multiplier=-1,
                        )
                    return pt, cs

                def emit_pv(kc, pt, cs, first, last):
                    for h in range(HPC):
                        nc.tensor.matmul(
                            psy[:D + 1, h, cs],
                            vaug[:, kc, h, :],
                            pt[:, h, cs],
                            start=first,
                            stop=last,
                        )

                # diagonal chunks: all scores first (their PVs are the
                # first psy writers and may briefly wait on the previous
                # j's psy release — keep the PE queue fed with scores)
                diag = kc_order[:4]
                diag_pt = [emit_scores(kc) for kc in diag]
                for i, kc in enumerate(diag):
                    pt, cs = diag_pt[i]
                    emit_pv(kc, pt, cs, i == 0, i == nkc - 1)
                for i, kc in enumerate(kc_order[4:]):
                    pt, cs = emit_scores(kc)
                    if fill_q:
                        fill_q.pop(0)()
                    emit_pv(kc, pt, cs, False, i == nkc - 5)

                # normalization, first half: drain psy fast on two
                # parallel engines (body rows -> yu on ACT, denom row 64 ->
                # sums on DVE) so the next j's PV can take the banks; the
                # partition broadcast and the multiplies are deferred one
                # boundary so the GpSimd queue never waits ahead of the
                # next j's causal selects.  sums lands on partition 0 — the
                # custom-DVE reciprocal mishandles offset partitions.
                yu = yu_pool.tile([D, HPC, TJ], F32, name="yu", tag="yu")
                nc.scalar.copy(yu[:], psy[:D, :, :])
                sums = sums_pool.tile([1, HPC, TJ], F32, name="sums",
                                      tag="sums")
                nc.vector.tensor_copy(out=sums[:], in_=psy[D:D + 1, :, :])
                recip = sums_pool.tile([1, HPC, TJ], F32, name="recip",
                                       tag="recip")
                nc.vector.reciprocal_approx_fast(out=recip[:], in_=sums[:])

                # ---- j-boundary fill work (keeps PE fed while the
                #      normalize chain for this j completes) ----
                while fill_q:
                    fill_q.pop(0)()
                new_fill = []
                for _ in range(4 if b == B - 1 else 2):
                    if pending_d:
                        args = pending_d.pop(0)
                        new_fill.append(lambda a=args: emit_d_pair(*a))
                if pending_norm:
                    emit_norm_finish(*pending_norm.pop(0))
                pending_norm.append((yu, recip, t0 + j * TJ))
                for _ in range((PACE0 if b == 0 else PACE)[j]):
                    if b_work:
                        tj, f = b_work.pop(0)
                        new_fill.append(
                            lambda tj=tj, f=f: emit_proj_group(
                                tj, f, x_tiles[b + 1], qkvts[b + 1],
                                vaugs[b + 1]))
                if b == 0:
                    # batch 0: boundary emission (prefetch arrives late)
                    for fn in new_fill:
                        fn()
                else:
                    fill_q = new_fill

            while fill_q:
                fill_q.pop(0)()
            while b_work:
                tj, f = b_work.pop(0)
                emit_proj_group(tj, f, x_tiles[b + 1], qkvts[b + 1],
                                vaugs[b + 1])
            if b + 2 < B:
                x_tiles[b + 2] = alloc_x_tiles()
                emit_prefetch(b + 2, x_tiles[b + 2])

        # tail: remaining normalize + output chunks of the last batch
        while pending_norm:
            emit_norm_finish(*pending_norm.pop(0))
        while pending_d:
            emit_d_pair(*pending_d.pop(0), split_drain=True)

    nc.compile()
    return nc


def make_in_maps(x, W_attn, b_attn, W_proj):
    x_flat = np.asarray(x, dtype=np.float32).reshape(NT, C)
    xt = np.ascontiguousarray(x_flat.T)
    xt_bf = xt.astype(ml_dtypes.bfloat16)
    xt_f8 = xt.astype(ml_dtypes.float8_e4m3)
    W_attn = np.asarray(W_attn, dtype=np.float32)
    b_attn = np.asarray(b_attn, dtype=np.float32)
    W_proj = np.asarray(W_proj, dtype=np.float32)
    in_maps = []
    for core in range(NCORES):
        lo = core * FC
        cols = np.concatenate(
            [np.arange(lo, lo + FC) + k * C for k in range(3)]
        )
        w_slice = W_attn[:, cols]
        wq, wk, wv = w_slice[:, :FC], w_slice[:, FC:2 * FC], w_slice[:, 2 * FC:]
        if Q_FP8:
            w_qv = wv
            w_k8 = np.concatenate([wk, wq], axis=1)
        else:
            w_qv = np.concatenate([wq, wv], axis=1)
            w_k8 = wk
        in_maps.append({
            "xt": xt_bf,
            "xt8": xt_f8,
            "w_qv": np.ascontiguousarray(w_qv.astype(ml_dtypes.bfloat16)),
            "w_k8": np.ascontiguousarray(
                (w_k8 * W_SCALE).astype(ml_dtypes.float8_e4m3)),
            "b_attn": np.ascontiguousarray(b_attn[cols].reshape(3, FC)),
            "w_proj": np.ascontiguousarray(
                W_proj[lo:lo + FC, :].astype(ml_dtypes.bfloat16)),
        })
    return in_maps


def kernel(x, W_attn, b_attn, W_proj, b_proj, **run_kwargs):
    if "nc" not in _CACHE:
        _CACHE["nc"] = build_program()
    nc = _CACHE["nc"]
    in_maps = make_in_maps(x, W_attn, b_attn, W_proj)
    res = run_bass_kernel_spmd(nc, in_maps, core_ids=list(range(NCORES)), **run_kwargs)
    _CACHE["last_results"] = res
    total = np.zeros((NT, C), dtype=np.float32)
    for r in res.results:
        total += np.asarray(r["out"], dtype=np.float32)
    total += np.asarray(b_proj, dtype=np.float32)[None, :]
    return total.reshape(B, T, C)
